# revision 8
# baseline (speedup 1.0000x reference)
"""TRN2 Bass kernel for nn_Conv1D_MEO (moe_routing).

Sharding: data-parallel over the B*N=64 segment axis -> core c owns sample c
(8 segments of 256 tokens). Gating, curve-merged expert weights (done once on
host, replicated), per-segment weight merge (PE, 4-seg x 4-expert packed
matmuls) and the batched GEMM run on device in fp32r.
"""
import sys
sys.path.insert(0, "/opt/trn_rl_repo")
import numpy as np
from contextlib import ExitStack

import concourse.bass as bass
import concourse.tile as tile
from concourse import bacc, mybir
from concourse.bass_utils import run_bass_kernel_spmd

F32 = mybir.dt.float32
F32R = mybir.dt.float32r

E, T = 8, 256
IN, OUT = 1024, 1024
D1 = D2 = DO1 = DO2 = 32
B, L = 8, 2048
N_SEG = L // T          # 8 segments per sample/core
NCORES = 8
LOSS_COEF = 1e-5


def _r11(x):
    """Round fp32 array to fp32r (11 mantissa bits, round-to-nearest-even)."""
    b = np.ascontiguousarray(x, dtype=np.float32).view(np.uint32)
    low = b & np.uint32((1 << 12) - 1)
    b2 = b & ~np.uint32((1 << 12) - 1)
    half = np.uint32(1 << 11)
    addone = (low > half) | ((low == half) & (((b2 >> 12) & 1) == 1))
    return (b2 + (addone.astype(np.uint32) << 12)).view(np.float32)


def _build_program():
    nc = bacc.Bacc("TRN2", target_bir_lowering=False, debug=False)

    xt_d = nc.dram_tensor("xt", [IN, L], F32, kind="ExternalInput").ap()
    rwp_d = nc.dram_tensor("rwp", [2, 32, 128, OUT], F32, kind="ExternalInput").ap()
    wg_d = nc.dram_tensor("wg", [IN, E], F32, kind="ExternalInput").ap()
    rb_d = nc.dram_tensor("rb", [E, OUT], F32, kind="ExternalInput").ap()
    i32b_d = nc.dram_tensor("i32b", [128, 32], F32, kind="ExternalInput").ap()
    id8_d = nc.dram_tensor("id8", [8, 8], F32, kind="ExternalInput").ap()

    y_d = nc.dram_tensor("y", [L, OUT], F32, kind="ExternalOutput").ap()
    gout_d = nc.dram_tensor("gates_o", [N_SEG, E], F32, kind="ExternalOutput").ap()
    gscr_d = nc.dram_tensor("gscr", [E, N_SEG], F32, kind="Internal").ap()
    ebscr_d = nc.dram_tensor("ebscr", [N_SEG, OUT], F32, kind="Internal").ap()

    with tile.TileContext(nc) as tc, ExitStack() as ctx:
        const = ctx.enter_context(tc.tile_pool(name="const", bufs=1))
        small = ctx.enter_context(tc.tile_pool(name="small", bufs=1))
        xstr = ctx.enter_context(tc.tile_pool(name="xstr", bufs=3))
        xrowp = ctx.enter_context(tc.tile_pool(name="xrowp", bufs=2))
        ebp = ctx.enter_context(tc.tile_pool(name="ebp", bufs=2))
        rwstr = ctx.enter_context(tc.tile_pool(name="rwstr", bufs=4))
        wpool = ctx.enter_context(tc.tile_pool(name="wpool", bufs=1))
        ypool = ctx.enter_context(tc.tile_pool(name="ypool", bufs=3))
        psA = ctx.enter_context(tc.tile_pool(name="psA", bufs=2, space="PSUM"))
        psB = ctx.enter_context(tc.tile_pool(name="psB", bufs=2, space="PSUM"))
        psS = ctx.enter_context(tc.tile_pool(name="psS", bufs=1, space="PSUM"))

        # ---------- constants ----------
        i32b = const.tile([128, 32], F32)
        nc.sync.dma_start(i32b[:], i32b_d[:])
        id8 = const.tile([8, 8], F32)
        nc.sync.dma_start(id8[:], id8_d[:])
        wgt = const.tile([128, 64], F32)   # [i-part, (i_tile, e)]
        nc.sync.dma_start(wgt[:], wg_d.rearrange("(t p) e -> p t e", p=128))
        rbt = const.tile([E, OUT], F32)
        nc.sync.dma_start(rbt[:], rb_d[:])
        ones_f = small.tile([1, 128], F32)
        nc.gpsimd.memset(ones_f[:], 1.0)
        ones_r = small.tile([1, 128], F32R)
        nc.vector.tensor_copy(ones_r[:], ones_f[:])

        # ---------- P1: x means (per segment) ----------
        means = small.tile([128, 64], F32)  # col = it*8 + s
        for it in range(8):
            xrow = xrowp.tile([128, L], F32, tag="xrow")
            nc.sync.dma_start(xrow[:], xt_d[it * 128:(it + 1) * 128, :])
            for s in range(N_SEG):
                nc.vector.reduce_sum(
                    out=means[:, it * 8 + s:it * 8 + s + 1],
                    in_=xrow[:, s * T:(s + 1) * T],
                    axis=mybir.AxisListType.X,
                )

        # ---------- P2: logits ----------
        pl = psS.tile([N_SEG, E], F32)
        for it in range(8):
            nc.tensor.matmul(
                pl[:], means[:, it * 8:it * 8 + 8], wgt[:, it * 8:it * 8 + 8],
                start=(it == 0), stop=(it == 7),
            )
        lg = small.tile([N_SEG, E], F32)
        nc.scalar.mul(lg[:], pl[:], 1.0 / T)

        # ---------- P3: softmax rows ----------
        mx = small.tile([N_SEG, 1], F32)
        nc.vector.reduce_max(out=mx[:], in_=lg[:], axis=mybir.AxisListType.X)
        sh = small.tile([N_SEG, E], F32)
        nc.vector.tensor_scalar(sh[:], lg[:], mx[:], None, op0=mybir.AluOpType.subtract)
        ex = small.tile([N_SEG, E], F32)
        sm = small.tile([N_SEG, 1], F32)
        nc.scalar.activation(ex[:], sh[:], mybir.ActivationFunctionType.Exp,
                             accum_out=sm[:])
        rc = small.tile([N_SEG, 1], F32)
        nc.vector.reciprocal(rc[:], sm[:])
        gates = small.tile([N_SEG, E], F32)
        nc.vector.tensor_scalar(gates[:], ex[:], rc[:], None, op0=mybir.AluOpType.mult)
        nc.sync.dma_start(gout_d[:], gates[:])

        # ---------- P4: gates^T, roll, to DRAM scratch ----------
        pgt = psS.tile([E, N_SEG], F32)
        nc.tensor.matmul(pgt[:], gates[:], id8[:], is_transpose=True)
        gT = small.tile([E, N_SEG], F32)
        nc.vector.tensor_copy(gT[:], pgt[:])
        gu = small.tile([E, N_SEG], F32)   # g_used^T: col s <- gates row (s-1 clamped)
        nc.vector.tensor_copy(gu[:, 1:8], gT[:, 0:7])
        nc.vector.tensor_copy(gu[:, 0:1], gT[:, 0:1])
        nc.sync.dma_start(gscr_d[:], gu[:])

        # ---------- P5: expert bias merge:  ebias = gu^T @ rb  ----------
        ebias8 = small.tile([N_SEG, OUT], F32R)
        for h in range(2):
            pb = psS.tile([N_SEG, 512], F32, tag="pb")
            nc.tensor.matmul(pb[:], gu[:], rbt[:, h * 512:(h + 1) * 512],
                             start=True, stop=True)
            nc.vector.tensor_copy(ebias8[:, h * 512:(h + 1) * 512], pb[:])
        nc.sync.dma_start(ebscr_d.bitcast(F32R), ebias8[:])

        # ---------- P5b: merge lhsT tiles ----------
        gb = {}
        for q in range(2):
            gbq = small.tile([128, 8], F32, tag=f"gb{q}")
            for ep in range(4):
                nc.sync.dma_start(
                    gbq[ep * 32:(ep + 1) * 32, :],
                    gscr_d[q * 4 + ep:q * 4 + ep + 1, :].partition_broadcast(32),
                )
            gb[q] = gbq
        lhsT = {}
        for g in range(2):
            for q in range(2):
                lt = small.tile([128, 128], F32R, tag=f"lhsT{g}{q}")
                for s in range(4):
                    nc.vector.tensor_scalar(
                        lt[:, s * 32:(s + 1) * 32], i32b[:],
                        gb[q][:, g * 4 + s:g * 4 + s + 1], None,
                        op0=mybir.AluOpType.mult,
                    )
                lhsT[(g, q)] = lt

        # ---------- P6: per o-half: merge weights then GEMM ----------
        wsb = []
        for s in range(8):
            wtile = wpool.tile([128, 8 * 512], F32R, tag=f"w{s}", name=f"wsb{s}")
            wsb.append(wtile)
        for h in range(2):
            for ib in range(32):
                it, sub = ib // 4, ib % 4
                rwq = []
                for q in range(2):
                    rt = rwstr.tile([128, 512], F32R, tag="rwq")
                    nc.sync.dma_start(
                        rt[:], rwp_d[q, ib, :, h * 512:(h + 1) * 512].bitcast(F32R))
                    rwq.append(rt)
                for g in range(2):
                    pw = psA.tile([128, 512], F32, tag="pw")
                    nc.tensor.matmul(pw[:], lhsT[(g, 0)][:], rwq[0][:], start=True, stop=False)
                    nc.tensor.matmul(pw[:], lhsT[(g, 1)][:], rwq[1][:],
                                     start=False, stop=True)
                    for s in range(4):
                        eng = nc.vector if s % 2 == 0 else nc.scalar
                        dst = wsb[g * 4 + s][sub * 32:(sub + 1) * 32,
                                             it * 512:(it + 1) * 512]
                        if s % 2 == 0:
                            nc.vector.tensor_copy(dst, pw[s * 32:(s + 1) * 32, :])
                        else:
                            nc.scalar.copy(dst, pw[s * 32:(s + 1) * 32, :])
            # GEMM for this o-half
            for s in range(8):
                ebias_s = ebp.tile([1, OUT], F32R, tag="ebias_s")
                nc.sync.dma_start(ebias_s[:], ebscr_d[s:s + 1, :].bitcast(F32R))
                xts = []
                for it in range(8):
                    xtile = xstr.tile([128, T], F32R, tag="xts")
                    nc.sync.dma_start(
                        xtile[:],
                        xt_d[it * 128:(it + 1) * 128, s * T:(s + 1) * T].bitcast(F32R))
                    xts.append(xtile)
                for tt in range(2):
                    py = psB.tile([128, 512], F32, tag="py")
                    for it in range(8):
                        nc.tensor.matmul(py[:], xts[it][:, tt * 128:(tt + 1) * 128],
                                         wsb[s][:, it * 512:(it + 1) * 512],
                                         start=(it == 0), stop=False)
                    nc.tensor.matmul(py[:], ones_r[:],
                                     ebias_s[0:1, h * 512:(h + 1) * 512],
                                     start=False, stop=True)
                    ysb = ypool.tile([128, 512], F32, tag="ysb")
                    if (s + tt) % 2 == 0:
                        nc.vector.tensor_copy(ysb[:], py[:])
                    else:
                        nc.scalar.copy(ysb[:], py[:])
                    nc.sync.dma_start(
                        y_d[s * T + tt * 128:s * T + (tt + 1) * 128,
                            h * 512:(h + 1) * 512], ysb[:])
    nc.compile()
    return nc


_NC = None


def kernel(x, w_gate, weight, bias, res_weight, res_bias,
           curve1_in, curve2_in, curve1_out, curve2_out,
           curve1_bias, curve2_bias):
    global _NC
    x = np.asarray(x, dtype=np.float32)
    w_gate = np.asarray(w_gate, dtype=np.float32)
    weight = np.asarray(weight, dtype=np.float32)
    bias = np.asarray(bias, dtype=np.float32)
    res_weight = np.asarray(res_weight, dtype=np.float32)
    res_bias = np.asarray(res_bias, dtype=np.float32)
    c1i = np.asarray(curve1_in, dtype=np.float32)
    c2i = np.asarray(curve2_in, dtype=np.float32)
    c1o = np.asarray(curve1_out, dtype=np.float32)
    c2o = np.asarray(curve2_out, dtype=np.float32)
    c1b = np.asarray(curve1_bias, dtype=np.float32)
    c2b = np.asarray(curve2_bias, dtype=np.float32)

    # ---- host: curve-factored weight merge (small relative to GEMM) ----
    rw = (weight - res_weight).reshape(E, DO1, DO2, D1, D2)
    rw = np.einsum("bij,bjklm->biklm", c1o, rw, optimize=True)
    rw = np.einsum("bik,bjklm->bjilm", c2o, rw, optimize=True)
    rw = np.einsum("bil,bjklm->bjkim", c1i, rw, optimize=True)
    rw = np.einsum("bim,bjklm->bjkli", c2i, rw, optimize=True)
    rw = rw.reshape(E, OUT, IN)
    rwp_full = rw + res_weight            # bake residual (gates sum to 1)

    rb = (bias - res_bias).reshape(E, DO1, DO2)
    rb = np.einsum("bki,bij->bkj", c1b, rb, optimize=True)
    rb = np.einsum("bkj,bij->bik", c2b, rb, optimize=True)
    rb = rb.reshape(E, OUT) + res_bias    # bake residual bias

    # pack merge rhs: [quad, i_blk, (e'*32+i''), o]
    rwT = np.ascontiguousarray(rwp_full.transpose(0, 2, 1))      # [e, i, o]
    rwp = np.empty((2, 32, 128, OUT), dtype=np.float32)
    for q in range(2):
        for ep in range(4):
            e = q * 4 + ep
            rwp[q, :, ep * 32:(ep + 1) * 32, :] = rwT[e].reshape(32, 32, OUT)
    rwp = _r11(rwp)

    i32b = np.zeros((128, 32), dtype=np.float32)
    for p in range(128):
        i32b[p, p % 32] = 1.0
    id8 = np.eye(8, dtype=np.float32)

    if _NC is None:
        _NC = _build_program()

    in_maps = []
    for c in range(NCORES):
        xt = _r11(np.ascontiguousarray(x[c].T))
        in_maps.append({
            "xt": xt, "rwp": rwp, "wg": w_gate, "rb": rb,
            "i32b": i32b, "id8": id8,
        })
    globals()["_LAST_IN_MAPS"] = in_maps
    res = run_bass_kernel_spmd(_NC, in_maps, core_ids=list(range(NCORES)))

    y = np.stack([res.results[c]["y"] for c in range(NCORES)], axis=0)
    gates_all = np.concatenate(
        [res.results[c]["gates_o"] for c in range(NCORES)], axis=0)  # [64, E]

    def cv_sq(v):
        v = v.astype(np.float64)
        return v.var(ddof=1) / (v.mean() ** 2 + 1e-10)

    importance = gates_all.sum(axis=0)
    load = (gates_all > 0).sum(axis=0).astype(np.float64)
    loss = np.float32((cv_sq(importance) + cv_sq(load)) * LOSS_COEF)
    return y, loss
